# revision 7
# baseline (speedup 1.0000x reference)
"""Trainium2 Bass kernel for ExpMemoryUpdater (scatter_memory).

Semantics (reference):
    mem_rows   = memory[unique_node_ids]                  # [n_upd, dim]
    decay      = exp((last_update[unique_node_ids] - timestamps) / LAMB)
    updated    = unique_messages + decay[:, None] * mem_rows
    updated_memory  = memory.at[unique_node_ids].set(updated)
    new_last_update = last_update.at[unique_node_ids].set(timestamps)

Distribution: the 100000 updated rows are row-sharded across 8 NeuronCores
(12500 rows each, padded to 12544 = 128*98). Each core computes its updated
rows; rows not touched by any update and the last_update scatter are pure
data placement, handled during host-side unshard/assembly.

On-core layout: partition p owns 98 consecutive rows of its shard, so every
DMA moves long contiguous per-partition runs (14 KB per partition per chunk,
1.75 MB per dma_start). decay is computed once as a [128, 98] tile; the row
update is a single fused DVE op per [128, 256] block:
    out = (mem * decay_scalar) + msg        (scalar_tensor_tensor)
"""

import sys
import types

import numpy as np

N_CORES = 8
N_NODES = 200000
DIM = 256
N_UPD = 100000
LAMB = 10.0

P = 128          # SBUF partitions
NBLK = 98        # row-blocks per partition
RPC = P * NBLK   # rows per core, padded (12544)
CH = 14          # row-blocks per DMA chunk
NCHUNK = NBLK // CH


def _install_ntff_hook_shim():
    """Make run_bass_kernel_spmd(trace=True) work under axon: the stock
    antenv package lacks axon_hooks, so register the ctypes NTFF hook from
    trn_agent_boot under that name."""
    try:
        import antenv.axon_hooks  # noqa: F401
        return
    except ImportError:
        pass
    try:
        import antenv
        from trn_agent_boot.trn_boot import _ntff_profile_via_ctypes

        mod = types.ModuleType("antenv.axon_hooks")
        hook = [_ntff_profile_via_ctypes("/opt/axon/libaxon_pjrt.so")]
        mod.set_axon_ntff_profile_hook = lambda h: hook.__setitem__(0, h)
        mod.get_axon_ntff_profile_hook = lambda: hook[0]
        sys.modules["antenv.axon_hooks"] = mod
        antenv.axon_hooks = mod
    except Exception:
        pass


_NC_CACHE = {}


def _build_nc():
    if "nc" in _NC_CACHE:
        return _NC_CACHE["nc"]

    import concourse.bacc as bacc
    import concourse.mybir as mybir
    from concourse import tile

    dt = mybir.dt.float32
    nc = bacc.Bacc(
        "TRN2", target_bir_lowering=False, debug=False, num_devices=N_CORES
    )
    mem = nc.dram_tensor("mem", [RPC, DIM], dt, kind="ExternalInput")
    msg = nc.dram_tensor("msg", [RPC, DIM], dt, kind="ExternalInput")
    lu = nc.dram_tensor("lu", [RPC], dt, kind="ExternalInput")
    ts = nc.dram_tensor("ts", [RPC], dt, kind="ExternalInput")
    out = nc.dram_tensor("out", [RPC, DIM], dt, kind="ExternalOutput")

    # Partition p owns rows [p*NBLK, (p+1)*NBLK): per-partition DRAM runs are
    # contiguous (NBLK rows * 1 KB), so chunked DMAs move CH KB contiguous
    # per partition.
    mem_v = mem[:].rearrange("(p n) d -> p n d", p=P)
    msg_v = msg[:].rearrange("(p n) d -> p n d", p=P)
    out_v = out[:].rearrange("(p n) d -> p n d", p=P)
    lu_v = lu[:].rearrange("(p n) -> p n", p=P)
    ts_v = ts[:].rearrange("(p n) -> p n", p=P)

    sub = mybir.AluOpType.subtract
    mult = mybir.AluOpType.mult
    add = mybir.AluOpType.add
    shl = mybir.AluOpType.arith_shift_left

    # Software exp to ~2 ulp (ACT's table Exp is only ~1e-5 relative):
    #   x = (lu - ts) * 0.1;  k = round(x * log2e)  (magic-add trick)
    #   u = x - k*ln2_hi - k*ln2_lo   (Cody-Waite, k*ln2_hi exact)
    #   exp(x) = 2^k * exp(u),  exp(u) via degree-7 Taylor
    # 2^k is built with one int op on the bitcast of the magic sum:
    # bits(y) = 0x4B400000 + k, so (bits(y) << 23) + 0x3F800000 == bits(2^k).
    import math
    INV_LAMB = float(np.float32(1.0 / LAMB))
    LOG2E = 1.4426950408889634
    MAGIC = 12582912.0  # 1.5 * 2^23
    LN2_HI = 0.693145751953125  # 0x3F317200, 9 trailing zero bits
    LN2_LO = 0.6931471805599453 - LN2_HI
    POLY = [1.0 / math.factorial(i) for i in range(8)]  # c0..c7

    with tile.TileContext(nc) as tc:
        with (
            tc.tile_pool(name="vec", bufs=1) as vpool,
            tc.tile_pool(name="memp", bufs=3) as mpool,
            tc.tile_pool(name="msgp", bufs=3) as spool,
        ):
            lu_t = vpool.tile([P, NBLK], dt)
            ts_t = vpool.tile([P, NBLK], dt)
            x_t = vpool.tile([P, NBLK], dt)
            y_t = vpool.tile([P, NBLK], dt)
            k_t = vpool.tile([P, NBLK], dt)
            u_t = vpool.tile([P, NBLK], dt)
            q_t = vpool.tile([P, NBLK], dt)
            sc_t = vpool.tile([P, NBLK], mybir.dt.int32)
            decay_t = vpool.tile([P, NBLK], dt)
            nc.sync.dma_start(out=lu_t[:], in_=lu_v)
            nc.sync.dma_start(out=ts_t[:], in_=ts_v)
            nc.vector.tensor_tensor(x_t[:], lu_t[:], ts_t[:], sub)
            nc.vector.tensor_scalar_mul(x_t[:], x_t[:], INV_LAMB)
            # y = x*log2e + MAGIC;  k = y - MAGIC  (round-to-nearest)
            nc.vector.tensor_scalar(
                out=y_t[:], in0=x_t[:], scalar1=LOG2E, scalar2=MAGIC,
                op0=mult, op1=add,
            )
            nc.vector.tensor_scalar_sub(k_t[:], y_t[:], MAGIC)
            # u = (k*(-ln2_hi) + x) + k*(-ln2_lo)
            nc.vector.scalar_tensor_tensor(
                out=u_t[:], in0=k_t[:], scalar=-LN2_HI, in1=x_t[:],
                op0=mult, op1=add,
            )
            nc.vector.scalar_tensor_tensor(
                out=u_t[:], in0=k_t[:], scalar=-LN2_LO, in1=u_t[:],
                op0=mult, op1=add,
            )
            # exp(u) = c0 + u*(c1 + u*(... + u*c7)) via q = (q + c)*u steps
            nc.vector.tensor_scalar_mul(q_t[:], u_t[:], POLY[7])
            for c_i in POLY[6:0:-1]:
                nc.vector.scalar_tensor_tensor(
                    out=q_t[:], in0=q_t[:], scalar=float(c_i), in1=u_t[:],
                    op0=add, op1=mult,
                )
            nc.vector.tensor_scalar_add(q_t[:], q_t[:], 1.0)
            nc.vector.tensor_scalar(
                out=sc_t[:], in0=y_t[:].bitcast(mybir.dt.int32),
                scalar1=23, scalar2=None, op0=shl,
            )
            nc.vector.tensor_scalar_add(sc_t[:], sc_t[:], 0x3F800000)
            nc.vector.tensor_tensor(
                decay_t[:], q_t[:], sc_t[:].bitcast(dt), mult
            )

            for c in range(NCHUNK):
                mem_t = mpool.tile([P, CH, DIM], dt)
                msg_t = spool.tile([P, CH, DIM], dt)
                nc.sync.dma_start(out=mem_t[:], in_=mem_v[:, c * CH:(c + 1) * CH, :])
                nc.sync.dma_start(out=msg_t[:], in_=msg_v[:, c * CH:(c + 1) * CH, :])
                for n in range(CH):
                    i = c * CH + n
                    nc.vector.scalar_tensor_tensor(
                        out=msg_t[:, n, :],
                        in0=mem_t[:, n, :],
                        scalar=decay_t[:, i:i + 1],
                        in1=msg_t[:, n, :],
                        op0=mult,
                        op1=add,
                    )
                nc.sync.dma_start(out=out_v[:, c * CH:(c + 1) * CH, :], in_=msg_t[:])

    nc.finalize()
    _NC_CACHE["nc"] = nc
    return nc


def _prep_in_maps(mem_rows, msgs, lu_rows, ts):
    """Pad the gathered update rows to 8*RPC and split per core."""
    n = mem_rows.shape[0]
    total = N_CORES * RPC
    mem_p = np.zeros((total, DIM), dtype=np.float32)
    msg_p = np.zeros((total, DIM), dtype=np.float32)
    lu_p = np.zeros(total, dtype=np.float32)
    ts_p = np.zeros(total, dtype=np.float32)
    mem_p[:n] = mem_rows
    msg_p[:n] = msgs
    lu_p[:n] = lu_rows
    ts_p[:n] = ts
    return [
        {
            "mem": mem_p[c * RPC:(c + 1) * RPC],
            "msg": msg_p[c * RPC:(c + 1) * RPC],
            "lu": lu_p[c * RPC:(c + 1) * RPC],
            "ts": ts_p[c * RPC:(c + 1) * RPC],
        }
        for c in range(N_CORES)
    ]


def _run_device(in_maps, trace=False):
    _install_ntff_hook_shim()
    from concourse.bass_utils import run_bass_kernel_spmd

    nc = _build_nc()
    return run_bass_kernel_spmd(
        nc, in_maps, list(range(N_CORES)), trace=trace
    )


def _updated_rows(res):
    """Concatenate per-core device outputs and strip padding."""
    outs = [res.results[c]["out"] for c in range(N_CORES)]
    return np.concatenate(outs, axis=0)[:N_UPD]


def kernel(memory, last_update, unique_node_ids, unique_messages, timestamps,
           _trace=False, _return_res=False):
    memory = np.asarray(memory)
    last_update = np.asarray(last_update)
    ids = np.asarray(unique_node_ids)
    msgs = np.asarray(unique_messages, dtype=np.float32)
    ts = np.asarray(timestamps, dtype=np.float32)
    n = ids.shape[0]

    contiguous = n == N_UPD and ids[0] == 0 and ids[-1] == n - 1 and np.array_equal(
        ids, np.arange(n, dtype=ids.dtype)
    )

    if contiguous:
        mem_rows = memory[:n]
        lu_rows = last_update[:n]
    else:
        mem_rows = memory[ids]
        lu_rows = last_update[ids]

    in_maps = _prep_in_maps(mem_rows, msgs, lu_rows, ts)
    res = _run_device(in_maps, trace=_trace)
    updated = _updated_rows(res)

    updated_memory = memory.copy()
    new_last_update = last_update.copy()
    if contiguous:
        updated_memory[:n] = updated
        new_last_update[:n] = ts
    else:
        updated_memory[ids] = updated
        new_last_update[ids] = ts

    if _return_res:
        return (updated_memory, new_last_update), res
    return updated_memory, new_last_update


# revision 10
# speedup vs baseline: 1.2342x; 1.2342x over previous
"""Trainium2 Bass kernel for ExpMemoryUpdater (scatter_memory).

Semantics (reference):
    mem_rows   = memory[unique_node_ids]                  # [n_upd, dim]
    decay      = exp((last_update[unique_node_ids] - timestamps) / LAMB)
    updated    = unique_messages + decay[:, None] * mem_rows
    updated_memory  = memory.at[unique_node_ids].set(updated)
    new_last_update = last_update.at[unique_node_ids].set(timestamps)

Distribution: the 100000 updated rows are row-sharded across 8 NeuronCores
(12500 rows each, padded to 12544 = 128*98). Each core computes its updated
rows; rows not touched by any update and the last_update scatter are pure
data placement, handled during host-side unshard/assembly.

On-core layout: partition p owns 98 consecutive rows of its shard, so every
DMA moves long contiguous per-partition runs (14 KB per partition per chunk,
1.75 MB per dma_start). decay is computed once as a [128, 98] tile; the row
update is a single fused DVE op per [128, 256] block:
    out = (mem * decay_scalar) + msg        (scalar_tensor_tensor)
"""

import sys
import types

import numpy as np

N_CORES = 8
N_NODES = 200000
DIM = 256
N_UPD = 100000
LAMB = 10.0

P = 128          # SBUF partitions
NBLK = 98        # row-blocks per partition
RPC = P * NBLK   # rows per core, padded (12544)
CH = 14          # row-blocks per DMA chunk
NCHUNK = NBLK // CH


def _install_ntff_hook_shim():
    """Make run_bass_kernel_spmd(trace=True) work under axon: the stock
    antenv package lacks axon_hooks, so register the ctypes NTFF hook from
    trn_agent_boot under that name."""
    try:
        import antenv.axon_hooks  # noqa: F401
        return
    except ImportError:
        pass
    try:
        import antenv
        from trn_agent_boot.trn_boot import _ntff_profile_via_ctypes

        mod = types.ModuleType("antenv.axon_hooks")
        hook = [_ntff_profile_via_ctypes("/opt/axon/libaxon_pjrt.so")]
        mod.set_axon_ntff_profile_hook = lambda h: hook.__setitem__(0, h)
        mod.get_axon_ntff_profile_hook = lambda: hook[0]
        sys.modules["antenv.axon_hooks"] = mod
        antenv.axon_hooks = mod
    except Exception:
        pass


_NC_CACHE = {}


def _build_nc():
    if "nc" in _NC_CACHE:
        return _NC_CACHE["nc"]

    import concourse.bacc as bacc
    import concourse.mybir as mybir
    from concourse import tile

    dt = mybir.dt.float32
    nc = bacc.Bacc(
        "TRN2", target_bir_lowering=False, debug=False, num_devices=N_CORES
    )
    mem = nc.dram_tensor("mem", [RPC, DIM], dt, kind="ExternalInput")
    msg = nc.dram_tensor("msg", [RPC, DIM], dt, kind="ExternalInput")
    lu = nc.dram_tensor("lu", [RPC], dt, kind="ExternalInput")
    ts = nc.dram_tensor("ts", [RPC], dt, kind="ExternalInput")
    out = nc.dram_tensor("out", [RPC, DIM], dt, kind="ExternalOutput")

    # Partition p owns rows [p*NBLK, (p+1)*NBLK): per-partition DRAM runs are
    # contiguous (NBLK rows * 1 KB), so chunked DMAs move CH KB contiguous
    # per partition.
    mem_v = mem[:].rearrange("(p n) d -> p n d", p=P)
    msg_v = msg[:].rearrange("(p n) d -> p n d", p=P)
    out_v = out[:].rearrange("(p n) d -> p n d", p=P)
    lu_v = lu[:].rearrange("(p n) -> p n", p=P)
    ts_v = ts[:].rearrange("(p n) -> p n", p=P)

    sub = mybir.AluOpType.subtract
    mult = mybir.AluOpType.mult
    add = mybir.AluOpType.add
    shl = mybir.AluOpType.arith_shift_left

    # Software exp to ~2 ulp (ACT's table Exp is only ~1e-5 relative):
    #   x = (lu - ts) * 0.1;  k = round(x * log2e)  (magic-add trick)
    #   u = x - k*ln2_hi - k*ln2_lo   (Cody-Waite, k*ln2_hi exact)
    #   exp(x) = 2^k * exp(u),  exp(u) via degree-7 Taylor
    # 2^k is built with one int op on the bitcast of the magic sum:
    # bits(y) = 0x4B400000 + k, so (bits(y) << 23) + 0x3F800000 == bits(2^k).
    import math
    INV_LAMB = float(np.float32(1.0 / LAMB))
    LOG2E = 1.4426950408889634
    MAGIC = 12582912.0  # 1.5 * 2^23
    LN2_HI = 0.693145751953125  # 0x3F317200, 9 trailing zero bits
    LN2_LO = 0.6931471805599453 - LN2_HI
    POLY = [1.0 / math.factorial(i) for i in range(8)]  # c0..c7

    with tile.TileContext(nc) as tc:
        with (
            tc.tile_pool(name="vec", bufs=1) as vpool,
            tc.tile_pool(name="memp", bufs=4) as mpool,
            tc.tile_pool(name="msgp", bufs=4) as spool,
        ):
            lu_t = vpool.tile([P, NBLK], dt)
            ts_t = vpool.tile([P, NBLK], dt)
            x_t = vpool.tile([P, NBLK], dt)
            y_t = vpool.tile([P, NBLK], dt)
            k_t = vpool.tile([P, NBLK], dt)
            u_t = vpool.tile([P, NBLK], dt)
            q_t = vpool.tile([P, NBLK], dt)
            sc_t = vpool.tile([P, NBLK], mybir.dt.int32)
            decay_t = vpool.tile([P, NBLK], dt)
            nc.sync.dma_start(out=lu_t[:], in_=lu_v)
            nc.sync.dma_start(out=ts_t[:], in_=ts_v)
            nc.vector.tensor_tensor(x_t[:], lu_t[:], ts_t[:], sub)
            nc.vector.tensor_scalar_mul(x_t[:], x_t[:], INV_LAMB)
            # y = x*log2e + MAGIC;  k = y - MAGIC  (round-to-nearest)
            nc.vector.tensor_scalar(
                out=y_t[:], in0=x_t[:], scalar1=LOG2E, scalar2=MAGIC,
                op0=mult, op1=add,
            )
            nc.vector.tensor_scalar_sub(k_t[:], y_t[:], MAGIC)
            # u = (k*(-ln2_hi) + x) + k*(-ln2_lo)
            nc.vector.scalar_tensor_tensor(
                out=u_t[:], in0=k_t[:], scalar=-LN2_HI, in1=x_t[:],
                op0=mult, op1=add,
            )
            nc.vector.scalar_tensor_tensor(
                out=u_t[:], in0=k_t[:], scalar=-LN2_LO, in1=u_t[:],
                op0=mult, op1=add,
            )
            # exp(u) = c0 + u*(c1 + u*(... + u*c7)) via q = (q + c)*u steps
            nc.vector.tensor_scalar_mul(q_t[:], u_t[:], POLY[7])
            for c_i in POLY[6:0:-1]:
                nc.vector.scalar_tensor_tensor(
                    out=q_t[:], in0=q_t[:], scalar=float(c_i), in1=u_t[:],
                    op0=add, op1=mult,
                )
            nc.vector.tensor_scalar_add(q_t[:], q_t[:], 1.0)
            nc.vector.tensor_scalar(
                out=sc_t[:], in0=y_t[:].bitcast(mybir.dt.int32),
                scalar1=23, scalar2=None, op0=shl,
            )
            nc.vector.tensor_scalar_add(sc_t[:], sc_t[:], 0x3F800000)
            nc.vector.tensor_tensor(
                decay_t[:], q_t[:], sc_t[:].bitcast(dt), mult
            )

            for c in range(NCHUNK):
                mem_t = mpool.tile([P, CH, DIM], dt)
                msg_t = spool.tile([P, CH, DIM], dt)
                nc.sync.dma_start(out=mem_t[:], in_=mem_v[:, c * CH:(c + 1) * CH, :])
                nc.sync.dma_start(out=msg_t[:], in_=msg_v[:, c * CH:(c + 1) * CH, :])
                # out = msg + decay*mem in two whole-chunk DVE ops; the decay
                # column block broadcasts along dim via a 0-stride AP.
                dec_b = decay_t[:, c * CH:(c + 1) * CH].broadcast_to([P, CH, DIM])
                nc.vector.tensor_tensor(mem_t[:], mem_t[:], dec_b, mult)
                nc.vector.tensor_tensor(msg_t[:], mem_t[:], msg_t[:], add)
                # Stores go on the ACT HWDGE ring so a store waiting on DVE
                # can't head-of-line-block later loads on the SP ring.
                nc.scalar.dma_start(out=out_v[:, c * CH:(c + 1) * CH, :], in_=msg_t[:])

    nc.finalize()
    _NC_CACHE["nc"] = nc
    return nc


def _prep_in_maps(mem_rows, msgs, lu_rows, ts):
    """Pad the gathered update rows to 8*RPC and split per core."""
    n = mem_rows.shape[0]
    total = N_CORES * RPC
    mem_p = np.zeros((total, DIM), dtype=np.float32)
    msg_p = np.zeros((total, DIM), dtype=np.float32)
    lu_p = np.zeros(total, dtype=np.float32)
    ts_p = np.zeros(total, dtype=np.float32)
    mem_p[:n] = mem_rows
    msg_p[:n] = msgs
    lu_p[:n] = lu_rows
    ts_p[:n] = ts
    return [
        {
            "mem": mem_p[c * RPC:(c + 1) * RPC],
            "msg": msg_p[c * RPC:(c + 1) * RPC],
            "lu": lu_p[c * RPC:(c + 1) * RPC],
            "ts": ts_p[c * RPC:(c + 1) * RPC],
        }
        for c in range(N_CORES)
    ]


def _run_device(in_maps, trace=False):
    _install_ntff_hook_shim()
    from concourse.bass_utils import run_bass_kernel_spmd

    nc = _build_nc()
    return run_bass_kernel_spmd(
        nc, in_maps, list(range(N_CORES)), trace=trace
    )


def _updated_rows(res):
    """Concatenate per-core device outputs and strip padding."""
    outs = [res.results[c]["out"] for c in range(N_CORES)]
    return np.concatenate(outs, axis=0)[:N_UPD]


def kernel(memory, last_update, unique_node_ids, unique_messages, timestamps,
           _trace=False, _return_res=False):
    memory = np.asarray(memory)
    last_update = np.asarray(last_update)
    ids = np.asarray(unique_node_ids)
    msgs = np.asarray(unique_messages, dtype=np.float32)
    ts = np.asarray(timestamps, dtype=np.float32)
    n = ids.shape[0]

    contiguous = n == N_UPD and ids[0] == 0 and ids[-1] == n - 1 and np.array_equal(
        ids, np.arange(n, dtype=ids.dtype)
    )

    if contiguous:
        mem_rows = memory[:n]
        lu_rows = last_update[:n]
    else:
        mem_rows = memory[ids]
        lu_rows = last_update[ids]

    in_maps = _prep_in_maps(mem_rows, msgs, lu_rows, ts)
    res = _run_device(in_maps, trace=_trace)
    updated = _updated_rows(res)

    updated_memory = memory.copy()
    new_last_update = last_update.copy()
    if contiguous:
        updated_memory[:n] = updated
        new_last_update[:n] = ts
    else:
        updated_memory[ids] = updated
        new_last_update[ids] = ts

    if _return_res:
        return (updated_memory, new_last_update), res
    return updated_memory, new_last_update


# revision 12
# speedup vs baseline: 1.2815x; 1.0383x over previous
"""Trainium2 Bass kernel for ExpMemoryUpdater (scatter_memory).

Semantics (reference):
    mem_rows   = memory[unique_node_ids]                  # [n_upd, dim]
    decay      = exp((last_update[unique_node_ids] - timestamps) / LAMB)
    updated    = unique_messages + decay[:, None] * mem_rows
    updated_memory  = memory.at[unique_node_ids].set(updated)
    new_last_update = last_update.at[unique_node_ids].set(timestamps)

Distribution: the 100000 updated rows are row-sharded across 8 NeuronCores
(12500 rows each, padded to 12544 = 128*98). Each core computes its updated
rows; rows not touched by any update and the last_update scatter are pure
data placement, handled during host-side unshard/assembly.

On-core layout: partition p owns 98 consecutive rows of its shard, so every
DMA moves long contiguous per-partition runs (14 KB per partition per chunk,
1.75 MB per dma_start). decay is computed once as a [128, 98] tile; the row
update is a single fused DVE op per [128, 256] block:
    out = (mem * decay_scalar) + msg        (scalar_tensor_tensor)
"""

import sys
import types

import numpy as np

N_CORES = 8
N_NODES = 200000
DIM = 256
N_UPD = 100000
LAMB = 10.0

P = 128          # SBUF partitions
NBLK = 98        # row-blocks per partition
RPC = P * NBLK   # rows per core, padded (12544)
# Row-blocks per DMA chunk. Large chunks amortize DMA setup; the final
# chunks shrink so the compute+store tail past the last load is short.
CHUNKS = [14, 14, 14, 14, 14, 14, 8, 4, 2]
assert sum(CHUNKS) == NBLK
CH_MAX = max(CHUNKS)


def _install_ntff_hook_shim():
    """Make run_bass_kernel_spmd(trace=True) work under axon: the stock
    antenv package lacks axon_hooks, so register the ctypes NTFF hook from
    trn_agent_boot under that name."""
    try:
        import antenv.axon_hooks  # noqa: F401
        return
    except ImportError:
        pass
    try:
        import antenv
        from trn_agent_boot.trn_boot import _ntff_profile_via_ctypes

        mod = types.ModuleType("antenv.axon_hooks")
        hook = [_ntff_profile_via_ctypes("/opt/axon/libaxon_pjrt.so")]
        mod.set_axon_ntff_profile_hook = lambda h: hook.__setitem__(0, h)
        mod.get_axon_ntff_profile_hook = lambda: hook[0]
        sys.modules["antenv.axon_hooks"] = mod
        antenv.axon_hooks = mod
    except Exception:
        pass


_NC_CACHE = {}


def _build_nc():
    if "nc" in _NC_CACHE:
        return _NC_CACHE["nc"]

    import concourse.bacc as bacc
    import concourse.mybir as mybir
    from concourse import tile

    dt = mybir.dt.float32
    nc = bacc.Bacc(
        "TRN2", target_bir_lowering=False, debug=False, num_devices=N_CORES
    )
    mem = nc.dram_tensor("mem", [RPC, DIM], dt, kind="ExternalInput")
    msg = nc.dram_tensor("msg", [RPC, DIM], dt, kind="ExternalInput")
    lu = nc.dram_tensor("lu", [RPC], dt, kind="ExternalInput")
    ts = nc.dram_tensor("ts", [RPC], dt, kind="ExternalInput")
    out = nc.dram_tensor("out", [RPC, DIM], dt, kind="ExternalOutput")

    # Partition p owns rows [p*NBLK, (p+1)*NBLK): per-partition DRAM runs are
    # contiguous (NBLK rows * 1 KB), so chunked DMAs move CH KB contiguous
    # per partition.
    mem_v = mem[:].rearrange("(p n) d -> p n d", p=P)
    msg_v = msg[:].rearrange("(p n) d -> p n d", p=P)
    out_v = out[:].rearrange("(p n) d -> p n d", p=P)
    lu_v = lu[:].rearrange("(p n) -> p n", p=P)
    ts_v = ts[:].rearrange("(p n) -> p n", p=P)

    sub = mybir.AluOpType.subtract
    mult = mybir.AluOpType.mult
    add = mybir.AluOpType.add
    shl = mybir.AluOpType.arith_shift_left

    # Software exp to ~2 ulp (ACT's table Exp is only ~1e-5 relative):
    #   x = (lu - ts) * 0.1;  k = round(x * log2e)  (magic-add trick)
    #   u = x - k*ln2_hi - k*ln2_lo   (Cody-Waite, k*ln2_hi exact)
    #   exp(x) = 2^k * exp(u),  exp(u) via degree-7 Taylor
    # 2^k is built with one int op on the bitcast of the magic sum:
    # bits(y) = 0x4B400000 + k, so (bits(y) << 23) + 0x3F800000 == bits(2^k).
    import math
    INV_LAMB = float(np.float32(1.0 / LAMB))
    LOG2E = 1.4426950408889634
    MAGIC = 12582912.0  # 1.5 * 2^23
    LN2_HI = 0.693145751953125  # 0x3F317200, 9 trailing zero bits
    LN2_LO = 0.6931471805599453 - LN2_HI
    POLY = [1.0 / math.factorial(i) for i in range(8)]  # c0..c7

    with tile.TileContext(nc) as tc:
        with (
            tc.tile_pool(name="vec", bufs=1) as vpool,
            tc.tile_pool(name="memp", bufs=4) as mpool,
            tc.tile_pool(name="msgp", bufs=4) as spool,
        ):
            lu_t = vpool.tile([P, NBLK], dt)
            ts_t = vpool.tile([P, NBLK], dt)
            x_t = vpool.tile([P, NBLK], dt)
            y_t = vpool.tile([P, NBLK], dt)
            k_t = vpool.tile([P, NBLK], dt)
            u_t = vpool.tile([P, NBLK], dt)
            q_t = vpool.tile([P, NBLK], dt)
            sc_t = vpool.tile([P, NBLK], mybir.dt.int32)
            decay_t = vpool.tile([P, NBLK], dt)
            nc.sync.dma_start(out=lu_t[:], in_=lu_v)
            nc.sync.dma_start(out=ts_t[:], in_=ts_v)
            nc.vector.tensor_tensor(x_t[:], lu_t[:], ts_t[:], sub)
            nc.vector.tensor_scalar_mul(x_t[:], x_t[:], INV_LAMB)
            # y = x*log2e + MAGIC;  k = y - MAGIC  (round-to-nearest)
            nc.vector.tensor_scalar(
                out=y_t[:], in0=x_t[:], scalar1=LOG2E, scalar2=MAGIC,
                op0=mult, op1=add,
            )
            nc.vector.tensor_scalar_sub(k_t[:], y_t[:], MAGIC)
            # u = (k*(-ln2_hi) + x) + k*(-ln2_lo)
            nc.vector.scalar_tensor_tensor(
                out=u_t[:], in0=k_t[:], scalar=-LN2_HI, in1=x_t[:],
                op0=mult, op1=add,
            )
            nc.vector.scalar_tensor_tensor(
                out=u_t[:], in0=k_t[:], scalar=-LN2_LO, in1=u_t[:],
                op0=mult, op1=add,
            )
            # exp(u) = c0 + u*(c1 + u*(... + u*c7)) via q = (q + c)*u steps
            nc.vector.tensor_scalar_mul(q_t[:], u_t[:], POLY[7])
            for c_i in POLY[6:0:-1]:
                nc.vector.scalar_tensor_tensor(
                    out=q_t[:], in0=q_t[:], scalar=float(c_i), in1=u_t[:],
                    op0=add, op1=mult,
                )
            nc.vector.tensor_scalar_add(q_t[:], q_t[:], 1.0)
            nc.vector.tensor_scalar(
                out=sc_t[:], in0=y_t[:].bitcast(mybir.dt.int32),
                scalar1=23, scalar2=None, op0=shl,
            )
            nc.vector.tensor_scalar_add(sc_t[:], sc_t[:], 0x3F800000)
            nc.vector.tensor_tensor(
                decay_t[:], q_t[:], sc_t[:].bitcast(dt), mult
            )

            base = 0
            for ch in CHUNKS:
                mem_t = mpool.tile([P, ch, DIM], dt, tag="memt")
                msg_t = spool.tile([P, ch, DIM], dt, tag="msgt")
                nc.sync.dma_start(out=mem_t[:], in_=mem_v[:, base:base + ch, :])
                nc.sync.dma_start(out=msg_t[:], in_=msg_v[:, base:base + ch, :])
                # out = msg + decay*mem in two whole-chunk DVE ops; the decay
                # column block broadcasts along dim via a 0-stride AP.
                dec_b = decay_t[:, base:base + ch].broadcast_to([P, ch, DIM])
                nc.vector.tensor_tensor(mem_t[:], mem_t[:], dec_b, mult)
                nc.vector.tensor_tensor(msg_t[:], mem_t[:], msg_t[:], add)
                # Stores go on the ACT HWDGE ring so a store waiting on DVE
                # can't head-of-line-block later loads on the SP ring.
                nc.scalar.dma_start(out=out_v[:, base:base + ch, :], in_=msg_t[:])
                base += ch

    nc.finalize()
    _NC_CACHE["nc"] = nc
    return nc


def _prep_in_maps(mem_rows, msgs, lu_rows, ts):
    """Pad the gathered update rows to 8*RPC and split per core."""
    n = mem_rows.shape[0]
    total = N_CORES * RPC
    mem_p = np.zeros((total, DIM), dtype=np.float32)
    msg_p = np.zeros((total, DIM), dtype=np.float32)
    lu_p = np.zeros(total, dtype=np.float32)
    ts_p = np.zeros(total, dtype=np.float32)
    mem_p[:n] = mem_rows
    msg_p[:n] = msgs
    lu_p[:n] = lu_rows
    ts_p[:n] = ts
    return [
        {
            "mem": mem_p[c * RPC:(c + 1) * RPC],
            "msg": msg_p[c * RPC:(c + 1) * RPC],
            "lu": lu_p[c * RPC:(c + 1) * RPC],
            "ts": ts_p[c * RPC:(c + 1) * RPC],
        }
        for c in range(N_CORES)
    ]


def _run_device(in_maps, trace=False):
    _install_ntff_hook_shim()
    from concourse.bass_utils import run_bass_kernel_spmd

    nc = _build_nc()
    return run_bass_kernel_spmd(
        nc, in_maps, list(range(N_CORES)), trace=trace
    )


def _updated_rows(res):
    """Concatenate per-core device outputs and strip padding."""
    outs = [res.results[c]["out"] for c in range(N_CORES)]
    return np.concatenate(outs, axis=0)[:N_UPD]


def kernel(memory, last_update, unique_node_ids, unique_messages, timestamps,
           _trace=False, _return_res=False):
    memory = np.asarray(memory)
    last_update = np.asarray(last_update)
    ids = np.asarray(unique_node_ids)
    msgs = np.asarray(unique_messages, dtype=np.float32)
    ts = np.asarray(timestamps, dtype=np.float32)
    n = ids.shape[0]

    contiguous = n == N_UPD and ids[0] == 0 and ids[-1] == n - 1 and np.array_equal(
        ids, np.arange(n, dtype=ids.dtype)
    )

    if contiguous:
        mem_rows = memory[:n]
        lu_rows = last_update[:n]
    else:
        mem_rows = memory[ids]
        lu_rows = last_update[ids]

    in_maps = _prep_in_maps(mem_rows, msgs, lu_rows, ts)
    res = _run_device(in_maps, trace=_trace)
    updated = _updated_rows(res)

    updated_memory = memory.copy()
    new_last_update = last_update.copy()
    if contiguous:
        updated_memory[:n] = updated
        new_last_update[:n] = ts
    else:
        updated_memory[ids] = updated
        new_last_update[ids] = ts

    if _return_res:
        return (updated_memory, new_last_update), res
    return updated_memory, new_last_update
